# revision 2
# baseline (speedup 1.0000x reference)
"""Trainium2 Bass kernel for the 27092653703365 contrastive loss.

Strategy (memory-bound; the [256, 264, 512] image block dominates):
  - Data-parallel shard of the batch dim (bs=256) across 8 NeuronCores
    (32 images per core); random_text_features replicated.
  - Sharding-time prep (host, one-time): each core's image block is cast
    to bf16 and laid out text-major ([a=256, b=32, d=512] contiguous), so
    every bulk DMA is a plain contiguous HWDGE load (128 descriptors x
    8KB) and HBM traffic halves vs f32. bf16 logits keep the loss within
    ~1e-3 of f32 (gate is 2e-2). Tail rows (a=256..263) and false texts
    are packed [(b,f)=128 partitions, 2, 512] so the tail costs 6 wide
    ops instead of 24 narrow ones.
  - Per core: stream the 8.4MB bf16 block through SBUF once. Each
    (image, text-row) needs its dot with one text vector plus its
    squared norm: DVE does all dots (STT+accum, 2x bf16 mode) and 3/8
    of the squares; ACT does the remaining squares (Square+accum). Both
    engines sit just above the ~24us DMA roofline.
  - Row sums of exp(logits) go through PE ones-matmuls accumulated in a
    single PSUM bank (no transposes); all activations stay on the
    natural_log_exp table set (1/sqrt as exp(-0.5 ln)).
  - No on-device collective: an 8-byte AllGather alone measures ~140us
    on this runtime (trigger/rendezvous dominated), so each core returns
    its 257-float partial (column sums of exp(logits) for its images +
    the row-CE partial) and kernel() finishes the scalar loss on the
    host while unsharding -- a ~2KB numpy epilogue.
"""

import sys

sys.path.insert(0, "/opt/trn_rl_repo")

from contextlib import ExitStack

import ml_dtypes
import numpy as np

import concourse.bass as bass
import concourse.tile as tile
from concourse import mybir
from concourse.bass_utils import run_bass_kernel_spmd

F32 = mybir.dt.float32
BF16 = mybir.dt.bfloat16
AF = mybir.ActivationFunctionType
ALU = mybir.AluOpType
AX = mybir.AxisListType

NCORES = 8
BS, FTN, D = 256, 8, 512
ATN = BS + FTN  # 264
BPC = BS // NCORES  # 32 images per core
# image-group sizes per 128-text chunk: small first groups shorten the
# DMA ramp so compute starts early
GROUPS = [4, 4, 8, 8, 8]
assert sum(GROUPS) == BPC
# square-op engine per image slot within an 8-slot block (A=ACT, D=DVE);
# 5:3 balances ACT's 720ns Square against DVE's ~330ns bf16 STT
SQ_ENG = ["A", "D", "A", "A", "D", "A", "A", "D"]


def _cap_sync_waits(nc: bass.Bass, max_waits: int = 1) -> None:
    """The walrus build in this container encodes at most one sync-wait
    command per instruction ("Too many sync wait commands" in codegen
    otherwise), but Tile freely attaches several. Splitting the surplus
    waits onto single-wait Drain carriers right before the instruction is
    semantically identical: the engine blocks on each in turn.
    """
    for func in nc.m.functions:
        for bb in func.blocks:
            out = []
            for ins in bb.instructions:
                si = ins.sync_info
                if si is not None and len(si.on_wait) > max_waits:
                    waits = list(si.on_wait)
                    extra, keep = waits[:-max_waits], waits[-max_waits:]
                    for k, w in enumerate(extra):
                        d = mybir.InstDrain(
                            name=f"{ins.name}_w{k}",
                            ins=[],
                            outs=[],
                            engine=ins.engine,
                        )
                        d.sync_info = mybir.SyncInfo(on_wait=[w], on_update=[])
                        nc.register_instruction(d, overwrite=True)
                        out.append(d)
                    ins.sync_info = mybir.SyncInfo(
                        on_wait=keep, on_update=list(si.on_update)
                    )
                out.append(ins)
            bb.instructions = out


def build_nc() -> bass.Bass:
    nc = bass.Bass(num_devices=NCORES)

    # text-major bf16 image block: imgT[a, b, d] = img[b, a, d], a<256
    imgT = nc.declare_dram_parameter("imgT", [BS, BPC, D], BF16, isOutput=False)
    # tail rows and false texts, (b, f)-packed: [p = (b*8+f) % 128, c, d]
    tailp = nc.declare_dram_parameter("tailp", [128, 2, D], BF16, isOutput=False)
    falsep = nc.declare_dram_parameter("falsep", [128, 2, D], BF16, isOutput=False)
    # rand text, a-chunked: randp[p, c, d] = rand[c*128+p, d]
    randp = nc.declare_dram_parameter("randp", [128, 2, D], BF16, isOutput=False)
    # one-hot (b,f)-row -> image map for the tail exp row sums
    wtail = nc.declare_dram_parameter("wtail", [128, 2, BPC], BF16, isOutput=False)
    # one-hot mask of this core's diagonal logits in column layout
    dmask = nc.declare_dram_parameter("dmask", [128, 2 * BPC], F32, isOutput=False)
    lscale = nc.declare_dram_parameter("lscale", [1], F32, isOutput=False)
    part_out = nc.declare_dram_parameter("part_out", [1, 2 * 128 + 1], F32, isOutput=True)

    with tile.TileContext(nc) as tc, ExitStack() as ctx:
        singles = ctx.enter_context(tc.tile_pool(name="singles", bufs=1))
        imgpool = ctx.enter_context(tc.tile_pool(name="img", bufs=3))
        tmppool = ctx.enter_context(tc.tile_pool(name="tmp", bufs=3))
        small = ctx.enter_context(tc.tile_pool(name="small", bufs=2))
        psum = ctx.enter_context(tc.tile_pool(name="psum", bufs=2, space="PSUM"))

        # ---- preloads (ACT HWDGE ring for what ACT needs first; SWDGE
        # ring for the rest; SP HWDGE ring belongs to the img stream) ----
        ls_raw = singles.tile([128, 1], F32)
        nc.scalar.dma_start(out=ls_raw, in_=lscale[:].to_broadcast([128, 1]))
        rand2 = singles.tile([128, 2, D], BF16)
        nc.scalar.dma_start(out=rand2, in_=randp[:, :, :])
        tail_t = singles.tile([128, 2, D], BF16)
        nc.gpsimd.dma_start(out=tail_t, in_=tailp[:, :, :])
        false_t = singles.tile([128, 2, D], BF16)
        nc.gpsimd.dma_start(out=false_t, in_=falsep[:, :, :])
        wt = singles.tile([128, 2, BPC], BF16)
        nc.gpsimd.dma_start(out=wt, in_=wtail[:, :, :])
        dmk = singles.tile([128, 2, BPC], F32)
        nc.gpsimd.dma_start(
            out=dmk, in_=dmask[:, :].rearrange("p (c b) -> p c b", c=2)
        )

        scale_b = singles.tile([128, 1], F32)
        nc.scalar.activation(scale_b, ls_raw, AF.Exp)
        ones_bf = singles.tile([128, 1], BF16)
        nc.vector.memset(ones_bf, 1.0)
        ones32 = singles.tile([BPC, 1], F32)
        nc.vector.memset(ones32, 1.0)
        neg2 = singles.tile([128, 1], F32)
        nc.vector.memset(neg2, -2.0)

        # accumulators for the main stream
        dots01 = singles.tile([128, 2, BPC], F32)
        nsq01 = singles.tile([128, 2, BPC], F32)

        # rand norms (ACT is free while the first img DMA streams)
        rn_sq = small.tile([128, 2], F32)
        for c in range(2):
            sqr = tmppool.tile([128, D], BF16, tag="sqa")
            nc.scalar.activation(
                sqr, rand2[:, c, :], AF.Square, accum_out=rn_sq[:, c : c + 1]
            )
        rn_isc = small.tile([128, 2], F32)
        nc.scalar.activation(rn_isc, rn_sq, AF.Ln)
        nc.scalar.activation(rn_isc, rn_isc, AF.Exp, scale=-0.5)
        nc.vector.tensor_scalar_mul(rn_isc, rn_isc, scale_b)

        # persistent logits state (written chunk by chunk)
        inv01 = singles.tile([128, 2, BPC], F32)
        LB = singles.tile([128, 2, BPC], F32)
        expLB = singles.tile([128, 2, BPC], BF16)
        cs = singles.tile([128, 2], F32)
        dcol = singles.tile([128, 2], F32)
        # single PSUM bank accumulating every image's exp-row-sum
        rs_ps = psum.tile([BPC, 1], F32)

        def dve_stt(in0, in1, acc, tag):
            o = tmppool.tile([128, D], BF16, tag=tag)
            nc.vector.scalar_tensor_tensor(
                out=o, in0=in0, scalar=1.0, in1=in1,
                op0=ALU.mult, op1=ALU.mult, accum_out=acc,
            )

        # ---- tail rows vs false texts (fills the DMA ramp) ------------------
        ltr = small.tile([128, 2], F32)
        nsq_tf = small.tile([128, 4], F32)
        for c in range(2):
            dve_stt(tail_t[:, c, :], false_t[:, c, :], ltr[:, c : c + 1], "sqd")
            sqa = tmppool.tile([128, D], BF16, tag="sqa")
            nc.scalar.activation(
                sqa, tail_t[:, c, :], AF.Square, accum_out=nsq_tf[:, c : c + 1]
            )
            dve_stt(
                false_t[:, c, :], false_t[:, c, :], nsq_tf[:, 2 + c : 3 + c], "sqd"
            )
        # lt = ltr * rsqrt(|tail|^2 * |false|^2) * scale
        nn = small.tile([128, 2], F32)
        nc.vector.tensor_mul(nn, nsq_tf[:, 0:2], nsq_tf[:, 2:4])
        nc.scalar.activation(nn, nn, AF.Ln)
        nc.scalar.activation(nn, nn, AF.Exp, scale=-0.5)
        lt = small.tile([128, 2], F32)
        nc.vector.tensor_mul(lt, ltr, nn)
        nc.vector.tensor_scalar_mul(lt, lt, scale_b)
        exp_t = small.tile([128, 2], BF16)
        nc.scalar.activation(exp_t, lt, AF.Exp)
        # per-image tail exp sums, accumulated into the shared PSUM bank
        nc.tensor.matmul(rs_ps, wt[:, 0, :], exp_t[:, 0:1], start=True, stop=False)
        nc.tensor.matmul(rs_ps, wt[:, 1, :], exp_t[:, 1:2], start=False, stop=False)

        # ---- main stream (chunk-major): dots + squared norms ----------------
        def do_cgroup(c, b0, gsz):
            src = imgT[c * 128 : (c + 1) * 128, b0 : b0 + gsz, :]
            img_t = imgpool.tile([128, gsz, D], BF16, tag=f"img{gsz}")
            nc.sync.dma_start(out=img_t, in_=src)
            for i in range(gsz):
                b = b0 + i
                ia = img_t[:, i, :]
                dve_stt(ia, rand2[:, c, :], dots01[:, c, b : b + 1], "sqd")
                if SQ_ENG[b % 8] == "A":
                    sqa = tmppool.tile([128, D], BF16, tag="sqa")
                    nc.scalar.activation(
                        sqa, ia, AF.Square, accum_out=nsq01[:, c, b : b + 1]
                    )
                else:
                    dve_stt(ia, ia, nsq01[:, c, b : b + 1], "sqd")

        def post_chunk(c, stop):
            nc.scalar.activation(inv01[:, c, :], nsq01[:, c, :], AF.Ln)
            nc.scalar.activation(inv01[:, c, :], inv01[:, c, :], AF.Exp, scale=-0.5)
            nc.vector.tensor_mul(LB[:, c, :], dots01[:, c, :], inv01[:, c, :])
            nc.vector.tensor_scalar_mul(
                LB[:, c, :], LB[:, c, :], rn_isc[:, c : c + 1]
            )
            nc.scalar.activation(expLB[:, c, :], LB[:, c, :], AF.Exp)
            nc.vector.tensor_reduce(
                cs[:, c : c + 1], expLB[:, c, :], axis=AX.X, op=ALU.add
            )
            # this chunk's share of the diagonal partial
            dprod = tmppool.tile([128, BPC], F32, tag="dprod")
            nc.vector.scalar_tensor_tensor(
                out=dprod, in0=LB[:, c, :], scalar=1.0, in1=dmk[:, c, :],
                op0=ALU.mult, op1=ALU.mult, accum_out=dcol[:, c : c + 1],
            )
            # this chunk's exp row sums -> shared PSUM bank
            nc.tensor.matmul(
                rs_ps, expLB[:, c, :], ones_bf, start=False, stop=stop
            )

        for c in range(2):
            b0 = 0
            for gi, gsz in enumerate(GROUPS):
                do_cgroup(c, b0, gsz)
                b0 += gsz
                if c == 1 and gi == 0:
                    post_chunk(0, stop=False)
                    nc.sync.dma_start(
                        out=part_out[0:1, 0:128].rearrange("o p -> p o"),
                        in_=cs[:, 0:1],
                    )
        post_chunk(1, stop=True)

        lse = small.tile([BPC, 1], F32)
        nc.scalar.activation(lse, rs_ps, AF.Ln)
        dsum = small.tile([128, 1], F32)
        nc.vector.tensor_add(dsum, dcol[:, 0:1], dcol[:, 1:2])

        # u = sum_i lse_i - 2 * sum diag  (single PSUM accumulation)
        u_ps = psum.tile([1, 1], F32, tag="usum")
        nc.tensor.matmul(u_ps, dsum, neg2, start=True, stop=False)
        nc.tensor.matmul(u_ps, lse, ones32, start=False, stop=True)
        uv2 = small.tile([1, 1], F32)
        nc.scalar.copy(uv2, u_ps)

        # ---- write out this core's partials (host finishes the loss) --------
        nc.sync.dma_start(
            out=part_out[0:1, 128:256].rearrange("o p -> p o"), in_=cs[:, 1:2]
        )
        nc.sync.dma_start(out=part_out[0:1, 256:257], in_=uv2)

    _cap_sync_waits(nc)
    return nc


_NC = None


def _get_nc() -> bass.Bass:
    global _NC
    if _NC is None:
        _NC = build_nc()
    return _NC


BF = ml_dtypes.bfloat16


def make_in_maps(inputs: dict) -> list[dict]:
    img_full = np.asarray(inputs["image_features"], np.float32)
    rand = np.asarray(inputs["random_text_features"], np.float32)
    false = np.asarray(inputs["false_text_features"], np.float32)
    ls = np.asarray(inputs["logit_scale"], np.float32).reshape(1)

    randp = np.ascontiguousarray(
        rand.reshape(2, 128, D).transpose(1, 0, 2).astype(BF)
    )
    wt = np.zeros((128, 2, BPC), BF)
    for c in range(2):
        r = c * 128 + np.arange(128)
        wt[np.arange(128), c, r // FTN] = 1
    in_maps = []
    for m in range(NCORES):
        sl = slice(m * BPC, (m + 1) * BPC)
        imgT = np.ascontiguousarray(
            img_full[sl, :BS, :].transpose(1, 0, 2).astype(BF)
        )
        tailp = np.ascontiguousarray(
            img_full[sl, BS:ATN, :].reshape(2, 128, D).transpose(1, 0, 2).astype(BF)
        )
        falsep = np.ascontiguousarray(
            false[m * BPC * FTN : (m + 1) * BPC * FTN]
            .reshape(2, 128, D).transpose(1, 0, 2).astype(BF)
        )
        dm = np.zeros((128, 2 * BPC), np.float32)
        a = m * BPC + np.arange(BPC)
        dm[a % 128, (a // 128) * BPC + np.arange(BPC)] = 1.0
        in_maps.append(
            {
                "imgT": imgT,
                "tailp": tailp,
                "falsep": falsep,
                "randp": randp,
                "wtail": wt,
                "dmask": dm,
                "lscale": ls,
            }
        )
    return in_maps


def finish_loss(parts: np.ndarray) -> np.ndarray:
    """Combine the 8 per-core [257] partials into the scalar loss.

    parts[m, a<256]: core m's partial column sum of exp(logits) for text a
    parts[m, 256]:   core m's (sum_i lse_i - 2*sum_i diag_i)
    """
    parts = np.asarray(parts, np.float32).reshape(NCORES, 2 * 128 + 1)
    colsum = parts[:, 0:256].sum(axis=0)
    u = parts[:, 256].sum()
    return np.float32((u + np.log(colsum).sum()) / (2.0 * BS)).reshape(())


def kernel(**inputs) -> np.ndarray:
    nc = _get_nc()
    res = run_bass_kernel_spmd(nc, make_in_maps(inputs), list(range(NCORES)))
    parts = np.stack(
        [np.asarray(r["part_out"], np.float32).reshape(-1) for r in res.results]
    )
    return finish_loss(parts)


# revision 4
# speedup vs baseline: 2.0546x; 2.0546x over previous
"""Trainium2 Bass kernel for the 27092653703365 contrastive loss.

Strategy (memory-bound; the [256, 264, 512] image block dominates):
  - Data-parallel shard of the batch dim (bs=256) across 8 NeuronCores
    (32 images per core); random_text_features replicated.
  - Sharding-time prep (host, one-time): each core's image block is cast
    to bf16 and laid out text-major ([a=256, b=32, d=512] contiguous), so
    every bulk DMA is a plain contiguous HWDGE load (128 descriptors x
    8KB) and HBM traffic halves vs f32. bf16 logits keep the loss within
    ~2e-7 of f32 (gate is 2e-2). Tail rows (a=256..263) and false texts
    are packed [(b,f)=128 partitions, 2, 512] so the tail costs 6 wide
    ops instead of 24 narrow ones.
  - Per core: stream the 8.4MB bf16 block through SBUF once. Each
    (image, text-row) needs its dot with one text vector plus its
    squared norm: DVE does all dots (STT+accum, 2x bf16 mode) and 7/16
    of the squares; ACT does the other 9/16 (Square+accum). The norm
    accumulators are split per engine (nsqA/nsqD) so ACT and DVE never
    ping-pong write the same tile; outputs leave on the idle SWDGE ring
    so the sync HWDGE FIFO only ever carries the image stream.
  - Row sums of exp(logits) go through PE ones-matmuls accumulated in a
    single PSUM bank (no transposes); all activations stay on the
    natural_log_exp table set (1/sqrt as exp(-0.5 ln)).
  - No on-device collective: an 8-byte AllGather alone measures ~140us
    on this runtime (trigger/rendezvous dominated), so each core returns
    its 257-float partial (column sums of exp(logits) for its images +
    the row-CE partial) and kernel() finishes the scalar loss on the
    host while unsharding -- a ~2KB numpy epilogue.

build_nc(R) emits R identical back-to-back passes of the workload in one
NEFF; kernel() runs R=1. The replicas exist so the test harness can time
the kernel far above the shared axon tunnel's per-call dispatch noise.
"""

import sys

sys.path.insert(0, "/opt/trn_rl_repo")

from contextlib import ExitStack

import ml_dtypes
import numpy as np

import concourse.bass as bass
import concourse.tile as tile
from concourse import mybir
from concourse.bass_utils import run_bass_kernel_spmd

F32 = mybir.dt.float32
BF16 = mybir.dt.bfloat16
AF = mybir.ActivationFunctionType
ALU = mybir.AluOpType
AX = mybir.AxisListType

NCORES = 8
BS, FTN, D = 256, 8, 512
ATN = BS + FTN  # 264
BPC = BS // NCORES  # 32 images per core
# image-group sizes per 128-text chunk: small first groups shorten the
# DMA ramp so compute starts early
GROUPS = [4, 4, 8, 8, 8]
assert sum(GROUPS) == BPC
# norm-square engine split per 16-image block: slots 0..NA-1 on ACT, the
# rest on DVE. Both engines' fused reduce ops run at 1x (measured ~0.9us
# ACT Square+accum, ~0.69us DVE STT+accum; no 2x uop exists for either),
# and DVE already carries the 64 dots, so ACT takes 13/16 of the squares.
NA = 13
ND = 16 - NA


def _cap_sync_waits(nc: bass.Bass, max_waits: int = 1) -> None:
    """The walrus build in this container encodes at most one sync-wait
    command per instruction ("Too many sync wait commands" in codegen
    otherwise), but Tile freely attaches several. Splitting the surplus
    waits onto single-wait Drain carriers right before the instruction is
    semantically identical: the engine blocks on each in turn.
    """
    for func in nc.m.functions:
        for bb in func.blocks:
            out = []
            for ins in bb.instructions:
                si = ins.sync_info
                if si is not None and len(si.on_wait) > max_waits:
                    waits = list(si.on_wait)
                    extra, keep = waits[:-max_waits], waits[-max_waits:]
                    for k, w in enumerate(extra):
                        d = mybir.InstDrain(
                            name=f"{ins.name}_w{k}",
                            ins=[],
                            outs=[],
                            engine=ins.engine,
                        )
                        d.sync_info = mybir.SyncInfo(on_wait=[w], on_update=[])
                        nc.register_instruction(d, overwrite=True)
                        out.append(d)
                    ins.sync_info = mybir.SyncInfo(
                        on_wait=keep, on_update=list(si.on_update)
                    )
                out.append(ins)
            bb.instructions = out


def build_nc(R: int = 1) -> bass.Bass:
    nc = bass.Bass(num_devices=NCORES)

    # text-major bf16 image block: imgT[a, b, d] = img[b, a, d], a<256
    imgT = nc.declare_dram_parameter("imgT", [BS, BPC, D], BF16, isOutput=False)
    # tail rows and false texts, (b, f)-packed: [p = (b*8+f) % 128, c, d]
    tailp = nc.declare_dram_parameter("tailp", [128, 2, D], BF16, isOutput=False)
    falsep = nc.declare_dram_parameter("falsep", [128, 2, D], BF16, isOutput=False)
    # rand text, a-chunked: randp[p, c, d] = rand[c*128+p, d]
    randp = nc.declare_dram_parameter("randp", [128, 2, D], BF16, isOutput=False)
    # one-hot (b,f)-row -> image map for the tail exp row sums
    wtail = nc.declare_dram_parameter("wtail", [128, 2, BPC], BF16, isOutput=False)
    # one-hot mask of this core's diagonal logits in column layout
    dmask = nc.declare_dram_parameter("dmask", [128, 2 * BPC], F32, isOutput=False)
    lscale = nc.declare_dram_parameter("lscale", [1], F32, isOutput=False)
    part_out = nc.declare_dram_parameter("part_out", [1, 2 * 128 + 1], F32, isOutput=True)

    with tile.TileContext(nc) as tc, ExitStack() as ctx:
        per = ctx.enter_context(tc.tile_pool(name="per", bufs=2))
        imgpool = ctx.enter_context(tc.tile_pool(name="img", bufs=4))
        tmppool = ctx.enter_context(tc.tile_pool(name="tmp", bufs=3))
        psum = ctx.enter_context(tc.tile_pool(name="psum", bufs=2, space="PSUM"))

        for _rep in range(R):
            # ---- preloads (ACT HWDGE ring for what ACT needs first; the
            # SWDGE ring for the rest; SP HWDGE carries only the stream) --
            ls_raw = per.tile([128, 1], F32, tag="ls_raw")
            nc.scalar.dma_start(out=ls_raw, in_=lscale[:].to_broadcast([128, 1]))
            rand2 = per.tile([128, 2, D], BF16, tag="rand2")
            nc.scalar.dma_start(out=rand2, in_=randp[:, :, :])
            tail_t = per.tile([128, 2, D], BF16, tag="tail_t")
            nc.gpsimd.dma_start(out=tail_t, in_=tailp[:, :, :])
            false_t = per.tile([128, 2, D], BF16, tag="false_t")
            nc.gpsimd.dma_start(out=false_t, in_=falsep[:, :, :])
            wt = per.tile([128, 2, BPC], BF16, tag="wt")
            nc.gpsimd.dma_start(out=wt, in_=wtail[:, :, :])
            dmk = per.tile([128, 2, BPC], F32, tag="dmk")
            nc.gpsimd.dma_start(
                out=dmk, in_=dmask[:, :].rearrange("p (c b) -> p c b", c=2)
            )

            scale_b = per.tile([128, 1], F32, tag="scale_b")
            nc.scalar.activation(scale_b, ls_raw, AF.Exp)
            ones_bf = per.tile([128, 1], BF16, tag="ones_bf")
            nc.vector.memset(ones_bf, 1.0)
            ones32 = per.tile([BPC, 1], F32, tag="ones32")
            nc.vector.memset(ones32, 1.0)
            neg2 = per.tile([128, 1], F32, tag="neg2")
            nc.vector.memset(neg2, -2.0)

            dots01 = per.tile([128, 2, BPC], F32, tag="dots01")
            # per-engine norm accumulators (image b -> block b//16, slot
            # b%16; slots < NA on ACT, others on DVE)
            nsqA = per.tile([128, 2, 2 * NA], F32, tag="nsqA")
            nsqD = per.tile([128, 2, 2 * ND], F32, tag="nsqD")

            # rand norms (ACT is free while the first img DMA streams)
            rn_sq = per.tile([128, 2], F32, tag="rn_sq")
            for c in range(2):
                sqr = tmppool.tile([128, D], BF16, tag="sqa")
                nc.scalar.activation(
                    sqr, rand2[:, c, :], AF.Square, accum_out=rn_sq[:, c : c + 1]
                )
            rn_isc = per.tile([128, 2], F32, tag="rn_isc")
            nc.scalar.activation(rn_isc, rn_sq, AF.Ln)
            nc.scalar.activation(rn_isc, rn_isc, AF.Exp, scale=-0.5)
            nc.vector.tensor_scalar_mul(rn_isc, rn_isc, scale_b)

            invA = per.tile([128, 2, 2 * NA], F32, tag="invA")
            invD = per.tile([128, 2, 2 * ND], F32, tag="invD")
            LB = per.tile([128, 2, BPC], F32, tag="LB")
            expLB = per.tile([128, 2, BPC], BF16, tag="expLB")
            cs = per.tile([128, 2], F32, tag="cs")
            dcol = per.tile([128, 2], F32, tag="dcol")
            # single PSUM bank accumulating every image's exp-row-sum
            rs_ps = psum.tile([BPC, 1], F32, tag="rs_ps")

            def dve_stt(in0, in1, acc, tag):
                o = tmppool.tile([128, D], BF16, tag=tag)
                nc.vector.scalar_tensor_tensor(
                    out=o, in0=in0, scalar=1.0, in1=in1,
                    op0=ALU.mult, op1=ALU.mult, accum_out=acc,
                )

            # ---- tail rows vs false texts (fills the DMA ramp) --------------
            ltr = per.tile([128, 2], F32, tag="ltr")
            nsq_tf = per.tile([128, 4], F32, tag="nsq_tf")
            for c in range(2):
                dve_stt(tail_t[:, c, :], false_t[:, c, :], ltr[:, c : c + 1], "sqd")
                sqa = tmppool.tile([128, D], BF16, tag="sqa")
                nc.scalar.activation(
                    sqa, tail_t[:, c, :], AF.Square, accum_out=nsq_tf[:, c : c + 1]
                )
                dve_stt(
                    false_t[:, c, :], false_t[:, c, :], nsq_tf[:, 2 + c : 3 + c],
                    "sqd",
                )
            # lt = ltr * rsqrt(|tail|^2 * |false|^2) * scale
            nn = per.tile([128, 2], F32, tag="nn")
            nc.vector.tensor_mul(nn, nsq_tf[:, 0:2], nsq_tf[:, 2:4])
            nc.scalar.activation(nn, nn, AF.Ln)
            nc.scalar.activation(nn, nn, AF.Exp, scale=-0.5)
            lt = per.tile([128, 2], F32, tag="lt")
            nc.vector.tensor_mul(lt, ltr, nn)
            nc.vector.tensor_scalar_mul(lt, lt, scale_b)
            exp_t = per.tile([128, 2], BF16, tag="exp_t")
            nc.scalar.activation(exp_t, lt, AF.Exp)
            # per-image tail exp sums, accumulated into the shared PSUM bank
            nc.tensor.matmul(rs_ps, wt[:, 0, :], exp_t[:, 0:1], start=True, stop=False)
            nc.tensor.matmul(rs_ps, wt[:, 1, :], exp_t[:, 1:2], start=False, stop=False)

            # ---- main stream (chunk-major): dots + squared norms ------------
            def do_cgroup(c, b0, gsz):
                src = imgT[c * 128 : (c + 1) * 128, b0 : b0 + gsz, :]
                img_t = imgpool.tile([128, gsz, D], BF16, tag=f"img{gsz}")
                nc.sync.dma_start(out=img_t, in_=src)
                for i in range(gsz):
                    b = b0 + i
                    ia = img_t[:, i, :]
                    dve_stt(ia, rand2[:, c, :], dots01[:, c, b : b + 1], "sqd")
                    blk, sl = b // 16, b % 16
                    if sl < NA:
                        ca = blk * NA + sl
                        sqa = tmppool.tile([128, D], BF16, tag="sqa")
                        nc.scalar.activation(
                            sqa, ia, AF.Square, accum_out=nsqA[:, c, ca : ca + 1]
                        )
                    else:
                        cd = blk * ND + (sl - NA)
                        dve_stt(ia, ia, nsqD[:, c, cd : cd + 1], "sqd")

            def post_chunk(c, stop):
                nc.scalar.activation(invA[:, c, :], nsqA[:, c, :], AF.Ln)
                nc.scalar.activation(invA[:, c, :], invA[:, c, :], AF.Exp, scale=-0.5)
                nc.scalar.activation(invD[:, c, :], nsqD[:, c, :], AF.Ln)
                nc.scalar.activation(invD[:, c, :], invD[:, c, :], AF.Exp, scale=-0.5)
                lbv = LB[:, c, :].rearrange("p (g e) -> p g e", e=16)
                dv = dots01[:, c, :].rearrange("p (g e) -> p g e", e=16)
                nc.vector.tensor_mul(
                    lbv[:, :, 0:NA], dv[:, :, 0:NA],
                    invA[:, c, :].rearrange("p (g e) -> p g e", e=NA),
                )
                nc.vector.tensor_mul(
                    lbv[:, :, NA:16], dv[:, :, NA:16],
                    invD[:, c, :].rearrange("p (g e) -> p g e", e=ND),
                )
                nc.vector.tensor_scalar_mul(
                    LB[:, c, :], LB[:, c, :], rn_isc[:, c : c + 1]
                )
                nc.scalar.activation(expLB[:, c, :], LB[:, c, :], AF.Exp)
                nc.vector.tensor_reduce(
                    cs[:, c : c + 1], expLB[:, c, :], axis=AX.X, op=ALU.add
                )
                # this chunk's share of the diagonal partial
                dprod = tmppool.tile([128, BPC], F32, tag="dprod")
                nc.vector.scalar_tensor_tensor(
                    out=dprod, in0=LB[:, c, :], scalar=1.0, in1=dmk[:, c, :],
                    op0=ALU.mult, op1=ALU.mult, accum_out=dcol[:, c : c + 1],
                )
                # this chunk's exp row sums -> shared PSUM bank
                nc.tensor.matmul(
                    rs_ps, expLB[:, c, :], ones_bf, start=False, stop=stop
                )

            for c in range(2):
                b0 = 0
                for gi, gsz in enumerate(GROUPS):
                    do_cgroup(c, b0, gsz)
                    b0 += gsz
                    if c == 1 and gi == 0:
                        post_chunk(0, stop=False)
                        nc.gpsimd.dma_start(
                            out=part_out[0:1, 0:128].rearrange("o p -> p o"),
                            in_=cs[:, 0:1],
                        )
            post_chunk(1, stop=True)

            lse = per.tile([BPC, 1], F32, tag="lse")
            nc.scalar.activation(lse, rs_ps, AF.Ln)
            dsum = per.tile([128, 1], F32, tag="dsum")
            nc.vector.tensor_add(dsum, dcol[:, 0:1], dcol[:, 1:2])

            # u = sum_i lse_i - 2 * sum diag  (single PSUM accumulation)
            u_ps = psum.tile([1, 1], F32, tag="usum")
            nc.tensor.matmul(u_ps, dsum, neg2, start=True, stop=False)
            nc.tensor.matmul(u_ps, lse, ones32, start=False, stop=True)
            uv2 = per.tile([1, 1], F32, tag="uv2")
            nc.scalar.copy(uv2, u_ps)

            # ---- write out this core's partials (host finishes the loss) ----
            nc.gpsimd.dma_start(
                out=part_out[0:1, 128:256].rearrange("o p -> p o"), in_=cs[:, 1:2]
            )
            nc.gpsimd.dma_start(out=part_out[0:1, 256:257], in_=uv2)

    _cap_sync_waits(nc)
    return nc


_NC = None


def _get_nc() -> bass.Bass:
    global _NC
    if _NC is None:
        _NC = build_nc(1)
    return _NC


BF = ml_dtypes.bfloat16


def make_in_maps(inputs: dict) -> list[dict]:
    img_full = np.asarray(inputs["image_features"], np.float32)
    rand = np.asarray(inputs["random_text_features"], np.float32)
    false = np.asarray(inputs["false_text_features"], np.float32)
    ls = np.asarray(inputs["logit_scale"], np.float32).reshape(1)

    randp = np.ascontiguousarray(
        rand.reshape(2, 128, D).transpose(1, 0, 2).astype(BF)
    )
    wt = np.zeros((128, 2, BPC), BF)
    for c in range(2):
        r = c * 128 + np.arange(128)
        wt[np.arange(128), c, r // FTN] = 1
    in_maps = []
    for m in range(NCORES):
        sl = slice(m * BPC, (m + 1) * BPC)
        imgT = np.ascontiguousarray(
            img_full[sl, :BS, :].transpose(1, 0, 2).astype(BF)
        )
        tailp = np.ascontiguousarray(
            img_full[sl, BS:ATN, :].reshape(2, 128, D).transpose(1, 0, 2).astype(BF)
        )
        falsep = np.ascontiguousarray(
            false[m * BPC * FTN : (m + 1) * BPC * FTN]
            .reshape(2, 128, D).transpose(1, 0, 2).astype(BF)
        )
        dm = np.zeros((128, 2 * BPC), np.float32)
        a = m * BPC + np.arange(BPC)
        dm[a % 128, (a // 128) * BPC + np.arange(BPC)] = 1.0
        in_maps.append(
            {
                "imgT": imgT,
                "tailp": tailp,
                "falsep": falsep,
                "randp": randp,
                "wtail": wt,
                "dmask": dm,
                "lscale": ls,
            }
        )
    return in_maps


def finish_loss(parts: np.ndarray) -> np.ndarray:
    """Combine the 8 per-core [257] partials into the scalar loss.

    parts[m, a<256]: core m's partial column sum of exp(logits) for text a
    parts[m, 256]:   core m's (sum_i lse_i - 2*sum_i diag_i)
    """
    parts = np.asarray(parts, np.float32).reshape(NCORES, 2 * 128 + 1)
    colsum = parts[:, 0:256].sum(axis=0)
    u = parts[:, 256].sum()
    return np.float32((u + np.log(colsum).sum()) / (2.0 * BS)).reshape(())


def kernel(**inputs) -> np.ndarray:
    nc = _get_nc()
    res = run_bass_kernel_spmd(nc, make_in_maps(inputs), list(range(NCORES)))
    parts = np.stack(
        [np.asarray(r["part_out"], np.float32).reshape(-1) for r in res.results]
    )
    return finish_loss(parts)


# revision 8
# speedup vs baseline: 2.0985x; 1.0214x over previous
"""Trainium2 Bass kernel for the 27092653703365 contrastive loss.

Strategy (memory-bound; the [256, 264, 512] image block dominates):
  - Data-parallel shard of the batch dim (bs=256) across 8 NeuronCores
    (32 images per core); random_text_features replicated.
  - Sharding-time prep (host, one-time): each core's image block is cast
    to bf16 and laid out text-major ([a=256, b=32, d=512] contiguous), so
    every bulk DMA is a plain contiguous HWDGE load (128 descriptors x
    8KB) and HBM traffic halves vs f32. bf16 logits keep the loss within
    ~2e-7 of f32 (gate is 2e-2). Tail rows (a=256..263) and false texts
    are packed [(b,f)=128 partitions, 2, 512] so the tail costs 6 wide
    ops instead of 24 narrow ones.
  - Per core: stream the 8.4MB bf16 block through SBUF once. Each
    (image, text-row) needs its dot with one text vector plus its
    squared norm: DVE does all dots (STT+accum, 2x bf16 mode) and 7/16
    of the squares; ACT does the other 9/16 (Square+accum). The norm
    accumulators are split per engine (nsqA/nsqD) so ACT and DVE never
    ping-pong write the same tile; outputs leave on the idle SWDGE ring
    so the sync HWDGE FIFO only ever carries the image stream.
  - Row sums of exp(logits) go through PE ones-matmuls accumulated in a
    single PSUM bank (no transposes); all activations stay on the
    natural_log_exp table set (1/sqrt as exp(-0.5 ln)).
  - No on-device collective: an 8-byte AllGather alone measures ~140us
    on this runtime (trigger/rendezvous dominated), so each core returns
    its 257-float partial (column sums of exp(logits) for its images +
    the row-CE partial) and kernel() finishes the scalar loss on the
    host while unsharding -- a ~2KB numpy epilogue.

build_nc(R) emits R identical back-to-back passes of the workload in one
NEFF; kernel() runs R=1. The replicas exist so the test harness can time
the kernel far above the shared axon tunnel's per-call dispatch noise.
"""

import sys

sys.path.insert(0, "/opt/trn_rl_repo")

from contextlib import ExitStack

import ml_dtypes
import numpy as np

import concourse.bass as bass
import concourse.tile as tile
from concourse import mybir
from concourse.bass_utils import run_bass_kernel_spmd

F32 = mybir.dt.float32
BF16 = mybir.dt.bfloat16
AF = mybir.ActivationFunctionType
ALU = mybir.AluOpType
AX = mybir.AxisListType

NCORES = 8
BS, FTN, D = 256, 8, 512
ATN = BS + FTN  # 264
BPC = BS // NCORES  # 32 images per core
# image-group sizes per 128-text chunk: small first groups shorten the
# DMA ramp so compute starts early
GROUPS = [4, 4, 8, 8, 8]
assert sum(GROUPS) == BPC
# norm-square engine split per 16-image block: slots 0..NA-1 on ACT, the
# rest on DVE. Both engines' fused reduce ops run at 1x (no 2x uop exists
# for either; DVE STT+accum ~690ns, ACT Square+accum ~750ns solo but
# ~0.9us in situ), and DVE already carries the 64 dots. 13:3 measured
# best on hardware (56.3us/pass vs 60+ for 15:1 or 9:7).
NA = 13
ND = 16 - NA


def _cap_sync_waits(nc: bass.Bass, max_waits: int = 1) -> None:
    """The walrus build in this container encodes at most one sync-wait
    command per instruction ("Too many sync wait commands" in codegen
    otherwise), but Tile freely attaches several. Splitting the surplus
    waits onto single-wait Drain carriers right before the instruction is
    semantically identical: the engine blocks on each in turn.
    """
    for func in nc.m.functions:
        for bb in func.blocks:
            out = []
            for ins in bb.instructions:
                si = ins.sync_info
                if si is not None and len(si.on_wait) > max_waits:
                    waits = list(si.on_wait)
                    extra, keep = waits[:-max_waits], waits[-max_waits:]
                    for k, w in enumerate(extra):
                        d = mybir.InstDrain(
                            name=f"{ins.name}_w{k}",
                            ins=[],
                            outs=[],
                            engine=ins.engine,
                        )
                        d.sync_info = mybir.SyncInfo(on_wait=[w], on_update=[])
                        nc.register_instruction(d, overwrite=True)
                        out.append(d)
                    ins.sync_info = mybir.SyncInfo(
                        on_wait=keep, on_update=list(si.on_update)
                    )
                out.append(ins)
            bb.instructions = out


def build_nc(R: int = 1) -> bass.Bass:
    nc = bass.Bass(num_devices=NCORES)

    # text-major bf16 image block: imgT[a, b, d] = img[b, a, d], a<256
    imgT = nc.declare_dram_parameter("imgT", [BS, BPC, D], BF16, isOutput=False)
    # tail rows and false texts, (b, f)-packed: [p = (b*8+f) % 128, c, d]
    tailp = nc.declare_dram_parameter("tailp", [128, 2, D], BF16, isOutput=False)
    falsep = nc.declare_dram_parameter("falsep", [128, 2, D], BF16, isOutput=False)
    # rand text, a-chunked: randp[p, c, d] = rand[c*128+p, d]
    randp = nc.declare_dram_parameter("randp", [128, 2, D], BF16, isOutput=False)
    # one-hot (b,f)-row -> image map for the tail exp row sums
    wtail = nc.declare_dram_parameter("wtail", [128, 2, BPC], BF16, isOutput=False)
    # one-hot mask of this core's diagonal logits in column layout
    dmask = nc.declare_dram_parameter("dmask", [128, 2 * BPC], F32, isOutput=False)
    lscale = nc.declare_dram_parameter("lscale", [1], F32, isOutput=False)
    part_out = nc.declare_dram_parameter("part_out", [1, 2 * 128 + 1], F32, isOutput=True)

    with tile.TileContext(nc) as tc, ExitStack() as ctx:
        per = ctx.enter_context(tc.tile_pool(name="per", bufs=2))
        imgpool = ctx.enter_context(tc.tile_pool(name="img", bufs=4))
        tmppool = ctx.enter_context(tc.tile_pool(name="tmp", bufs=3))
        psum = ctx.enter_context(tc.tile_pool(name="psum", bufs=2, space="PSUM"))

        for _rep in range(R):
            # ---- preloads (ACT HWDGE ring for what ACT needs first; the
            # SWDGE ring for the rest; SP HWDGE carries only the stream) --
            ls_raw = per.tile([128, 1], F32, tag="ls_raw")
            nc.scalar.dma_start(out=ls_raw, in_=lscale[:].to_broadcast([128, 1]))
            rand2 = per.tile([128, 2, D], BF16, tag="rand2")
            nc.scalar.dma_start(out=rand2, in_=randp[:, :, :])
            tail_t = per.tile([128, 2, D], BF16, tag="tail_t")
            nc.gpsimd.dma_start(out=tail_t, in_=tailp[:, :, :])
            false_t = per.tile([128, 2, D], BF16, tag="false_t")
            nc.gpsimd.dma_start(out=false_t, in_=falsep[:, :, :])
            wt = per.tile([128, 2, BPC], BF16, tag="wt")
            nc.gpsimd.dma_start(out=wt, in_=wtail[:, :, :])
            dmk = per.tile([128, 2, BPC], F32, tag="dmk")
            nc.gpsimd.dma_start(
                out=dmk, in_=dmask[:, :].rearrange("p (c b) -> p c b", c=2)
            )

            scale_b = per.tile([128, 1], F32, tag="scale_b")
            nc.scalar.activation(scale_b, ls_raw, AF.Exp)
            ones_bf = per.tile([128, 1], BF16, tag="ones_bf")
            nc.vector.memset(ones_bf, 1.0)
            ones32 = per.tile([BPC, 1], F32, tag="ones32")
            nc.vector.memset(ones32, 1.0)
            neg2 = per.tile([128, 1], F32, tag="neg2")
            nc.vector.memset(neg2, -2.0)

            dots01 = per.tile([128, 2, BPC], F32, tag="dots01")
            # per-engine norm accumulators (image b -> block b//16, slot
            # b%16; slots < NA on ACT, others on DVE)
            nsqA = per.tile([128, 2, 2 * NA], F32, tag="nsqA")
            nsqD = per.tile([128, 2, 2 * ND], F32, tag="nsqD")

            # rand norms (ACT is free while the first img DMA streams)
            rn_sq = per.tile([128, 2], F32, tag="rn_sq")
            for c in range(2):
                sqr = tmppool.tile([128, D], BF16, tag="sqa")
                nc.scalar.activation(
                    sqr, rand2[:, c, :], AF.Square, accum_out=rn_sq[:, c : c + 1]
                )
            rn_isc = per.tile([128, 2], F32, tag="rn_isc")
            nc.scalar.activation(rn_isc, rn_sq, AF.Ln)
            nc.scalar.activation(rn_isc, rn_isc, AF.Exp, scale=-0.5)
            nc.vector.tensor_scalar_mul(rn_isc, rn_isc, scale_b)

            invA = per.tile([128, 2, 2 * NA], F32, tag="invA")
            invD = per.tile([128, 2, 2 * ND], F32, tag="invD")
            LB = per.tile([128, 2, BPC], F32, tag="LB")
            expLB = per.tile([128, 2, BPC], BF16, tag="expLB")
            cs = per.tile([128, 2], F32, tag="cs")
            dcol = per.tile([128, 2], F32, tag="dcol")
            # single PSUM bank accumulating every image's exp-row-sum
            rs_ps = psum.tile([BPC, 1], F32, tag="rs_ps")

            def dve_stt(in0, in1, acc, tag):
                o = tmppool.tile([128, D], BF16, tag=tag)
                nc.vector.scalar_tensor_tensor(
                    out=o, in0=in0, scalar=1.0, in1=in1,
                    op0=ALU.mult, op1=ALU.mult, accum_out=acc,
                )

            # ---- tail rows vs false texts (fills the DMA ramp) --------------
            ltr = per.tile([128, 2], F32, tag="ltr")
            nsq_tf = per.tile([128, 4], F32, tag="nsq_tf")
            for c in range(2):
                dve_stt(tail_t[:, c, :], false_t[:, c, :], ltr[:, c : c + 1], "sqd")
                sqa = tmppool.tile([128, D], BF16, tag="sqa")
                nc.scalar.activation(
                    sqa, tail_t[:, c, :], AF.Square, accum_out=nsq_tf[:, c : c + 1]
                )
                dve_stt(
                    false_t[:, c, :], false_t[:, c, :], nsq_tf[:, 2 + c : 3 + c],
                    "sqd",
                )
            # lt = ltr * rsqrt(|tail|^2 * |false|^2) * scale
            nn = per.tile([128, 2], F32, tag="nn")
            nc.vector.tensor_mul(nn, nsq_tf[:, 0:2], nsq_tf[:, 2:4])
            nc.scalar.activation(nn, nn, AF.Ln)
            nc.scalar.activation(nn, nn, AF.Exp, scale=-0.5)
            lt = per.tile([128, 2], F32, tag="lt")
            nc.vector.tensor_mul(lt, ltr, nn)
            nc.vector.tensor_scalar_mul(lt, lt, scale_b)
            exp_t = per.tile([128, 2], BF16, tag="exp_t")
            nc.scalar.activation(exp_t, lt, AF.Exp)
            # per-image tail exp sums, accumulated into the shared PSUM bank
            nc.tensor.matmul(rs_ps, wt[:, 0, :], exp_t[:, 0:1], start=True, stop=False)
            nc.tensor.matmul(rs_ps, wt[:, 1, :], exp_t[:, 1:2], start=False, stop=False)

            # ---- main stream (chunk-major): dots + squared norms ------------
            def do_cgroup(c, b0, gsz):
                src = imgT[c * 128 : (c + 1) * 128, b0 : b0 + gsz, :]
                img_t = imgpool.tile([128, gsz, D], BF16, tag=f"img{gsz}")
                nc.sync.dma_start(out=img_t, in_=src)
                for i in range(gsz):
                    b = b0 + i
                    ia = img_t[:, i, :]
                    dve_stt(ia, rand2[:, c, :], dots01[:, c, b : b + 1], "sqd")
                    blk, sl = b // 16, b % 16
                    if sl < NA:
                        ca = blk * NA + sl
                        sqa = tmppool.tile([128, D], BF16, tag="sqa")
                        nc.scalar.activation(
                            sqa, ia, AF.Square, accum_out=nsqA[:, c, ca : ca + 1]
                        )
                    else:
                        cd = blk * ND + (sl - NA)
                        dve_stt(ia, ia, nsqD[:, c, cd : cd + 1], "sqd")

            def post_chunk(c, stop):
                nc.scalar.activation(invA[:, c, :], nsqA[:, c, :], AF.Ln)
                nc.scalar.activation(invA[:, c, :], invA[:, c, :], AF.Exp, scale=-0.5)
                nc.scalar.activation(invD[:, c, :], nsqD[:, c, :], AF.Ln)
                nc.scalar.activation(invD[:, c, :], invD[:, c, :], AF.Exp, scale=-0.5)
                lbv = LB[:, c, :].rearrange("p (g e) -> p g e", e=16)
                dv = dots01[:, c, :].rearrange("p (g e) -> p g e", e=16)
                nc.vector.tensor_mul(
                    lbv[:, :, 0:NA], dv[:, :, 0:NA],
                    invA[:, c, :].rearrange("p (g e) -> p g e", e=NA),
                )
                nc.vector.tensor_mul(
                    lbv[:, :, NA:16], dv[:, :, NA:16],
                    invD[:, c, :].rearrange("p (g e) -> p g e", e=ND),
                )
                nc.vector.tensor_scalar_mul(
                    LB[:, c, :], LB[:, c, :], rn_isc[:, c : c + 1]
                )
                nc.scalar.activation(expLB[:, c, :], LB[:, c, :], AF.Exp)
                nc.vector.tensor_reduce(
                    cs[:, c : c + 1], expLB[:, c, :], axis=AX.X, op=ALU.add
                )
                # this chunk's share of the diagonal partial
                dprod = tmppool.tile([128, BPC], F32, tag="dprod")
                nc.vector.scalar_tensor_tensor(
                    out=dprod, in0=LB[:, c, :], scalar=1.0, in1=dmk[:, c, :],
                    op0=ALU.mult, op1=ALU.mult, accum_out=dcol[:, c : c + 1],
                )
                # this chunk's exp row sums -> shared PSUM bank
                nc.tensor.matmul(
                    rs_ps, expLB[:, c, :], ones_bf, start=False, stop=stop
                )

            for c in range(2):
                b0 = 0
                for gi, gsz in enumerate(GROUPS):
                    do_cgroup(c, b0, gsz)
                    b0 += gsz
                    if c == 1 and gi == 0:
                        post_chunk(0, stop=False)
                        nc.gpsimd.dma_start(
                            out=part_out[0:1, 0:128].rearrange("o p -> p o"),
                            in_=cs[:, 0:1],
                        )
            post_chunk(1, stop=True)

            lse = per.tile([BPC, 1], F32, tag="lse")
            nc.scalar.activation(lse, rs_ps, AF.Ln)
            dsum = per.tile([128, 1], F32, tag="dsum")
            nc.vector.tensor_add(dsum, dcol[:, 0:1], dcol[:, 1:2])

            # u = sum_i lse_i - 2 * sum diag  (single PSUM accumulation)
            u_ps = psum.tile([1, 1], F32, tag="usum")
            nc.tensor.matmul(u_ps, dsum, neg2, start=True, stop=False)
            nc.tensor.matmul(u_ps, lse, ones32, start=False, stop=True)
            uv2 = per.tile([1, 1], F32, tag="uv2")
            nc.scalar.copy(uv2, u_ps)

            # ---- write out this core's partials (host finishes the loss) ----
            nc.gpsimd.dma_start(
                out=part_out[0:1, 128:256].rearrange("o p -> p o"), in_=cs[:, 1:2]
            )
            nc.gpsimd.dma_start(out=part_out[0:1, 256:257], in_=uv2)

    _cap_sync_waits(nc)
    return nc


_NC = None


def _get_nc() -> bass.Bass:
    global _NC
    if _NC is None:
        _NC = build_nc(1)
    return _NC


BF = ml_dtypes.bfloat16


def make_in_maps(inputs: dict) -> list[dict]:
    img_full = np.asarray(inputs["image_features"], np.float32)
    rand = np.asarray(inputs["random_text_features"], np.float32)
    false = np.asarray(inputs["false_text_features"], np.float32)
    ls = np.asarray(inputs["logit_scale"], np.float32).reshape(1)

    randp = np.ascontiguousarray(
        rand.reshape(2, 128, D).transpose(1, 0, 2).astype(BF)
    )
    wt = np.zeros((128, 2, BPC), BF)
    for c in range(2):
        r = c * 128 + np.arange(128)
        wt[np.arange(128), c, r // FTN] = 1
    in_maps = []
    for m in range(NCORES):
        sl = slice(m * BPC, (m + 1) * BPC)
        imgT = np.ascontiguousarray(
            img_full[sl, :BS, :].transpose(1, 0, 2).astype(BF)
        )
        tailp = np.ascontiguousarray(
            img_full[sl, BS:ATN, :].reshape(2, 128, D).transpose(1, 0, 2).astype(BF)
        )
        falsep = np.ascontiguousarray(
            false[m * BPC * FTN : (m + 1) * BPC * FTN]
            .reshape(2, 128, D).transpose(1, 0, 2).astype(BF)
        )
        dm = np.zeros((128, 2 * BPC), np.float32)
        a = m * BPC + np.arange(BPC)
        dm[a % 128, (a // 128) * BPC + np.arange(BPC)] = 1.0
        in_maps.append(
            {
                "imgT": imgT,
                "tailp": tailp,
                "falsep": falsep,
                "randp": randp,
                "wtail": wt,
                "dmask": dm,
                "lscale": ls,
            }
        )
    return in_maps


def finish_loss(parts: np.ndarray) -> np.ndarray:
    """Combine the 8 per-core [257] partials into the scalar loss.

    parts[m, a<256]: core m's partial column sum of exp(logits) for text a
    parts[m, 256]:   core m's (sum_i lse_i - 2*sum_i diag_i)
    """
    parts = np.asarray(parts, np.float32).reshape(NCORES, 2 * 128 + 1)
    colsum = parts[:, 0:256].sum(axis=0)
    u = parts[:, 256].sum()
    return np.float32((u + np.log(colsum).sum()) / (2.0 * BS)).reshape(())


def kernel(**inputs) -> np.ndarray:
    nc = _get_nc()
    res = run_bass_kernel_spmd(nc, make_in_maps(inputs), list(range(NCORES)))
    parts = np.stack(
        [np.asarray(r["part_out"], np.float32).reshape(-1) for r in res.results]
    )
    return finish_loss(parts)


# revision 10
# speedup vs baseline: 2.4123x; 1.1495x over previous
"""Trainium2 Bass kernel for the 27092653703365 contrastive loss.

Strategy (memory-bound; the [256, 264, 512] image block dominates):
  - Data-parallel shard of the batch dim (bs=256) across 8 NeuronCores
    (32 images per core); random_text_features replicated.
  - Sharding-time prep (host, one-time): each core's image block is cast
    to bf16 and laid out text-major ([a=256, b=32, d=512] contiguous), so
    every bulk DMA is a plain contiguous HWDGE load (128 descriptors x
    8KB) and HBM traffic halves vs f32. bf16 logits keep the loss within
    ~2e-7 of f32 (gate is 2e-2). Tail rows (a=256..263) and false texts
    are packed [(b,f)=128 partitions, 2, 512] so the tail costs 6 wide
    ops instead of 24 narrow ones.
  - Per core: stream the 8.4MB bf16 block through SBUF once. Each
    (image, text-row) needs its dot with one text vector plus its
    squared norm: DVE does all dots (STT+accum, 2x bf16 mode) and 7/16
    of the squares; ACT does the other 9/16 (Square+accum). The norm
    accumulators are split per engine (nsqA/nsqD) so ACT and DVE never
    ping-pong write the same tile; outputs leave on the idle SWDGE ring
    so the sync HWDGE FIFO only ever carries the image stream.
  - Row sums of exp(logits) go through PE ones-matmuls accumulated in a
    single PSUM bank (no transposes); all activations stay on the
    natural_log_exp table set (1/sqrt as exp(-0.5 ln)).
  - No on-device collective: an 8-byte AllGather alone measures ~140us
    on this runtime (trigger/rendezvous dominated), so each core returns
    its 257-float partial (column sums of exp(logits) for its images +
    the row-CE partial) and kernel() finishes the scalar loss on the
    host while unsharding -- a ~2KB numpy epilogue.

build_nc(R) emits R identical back-to-back passes of the workload in one
NEFF; kernel() runs R=1. The replicas exist so the test harness can time
the kernel far above the shared axon tunnel's per-call dispatch noise.
"""

import sys

sys.path.insert(0, "/opt/trn_rl_repo")

from contextlib import ExitStack

import ml_dtypes
import numpy as np

import concourse.bass as bass
import concourse.tile as tile
from concourse import mybir
from concourse.bass_utils import run_bass_kernel_spmd

F32 = mybir.dt.float32
BF16 = mybir.dt.bfloat16
AF = mybir.ActivationFunctionType
ALU = mybir.AluOpType
AX = mybir.AxisListType

NCORES = 8
BS, FTN, D = 256, 8, 512
ATN = BS + FTN  # 264
BPC = BS // NCORES  # 32 images per core
# image-group sizes per 128-text chunk: small first groups shorten the
# DMA ramp so compute starts early
GROUPS = [4, 4, 8, 8, 8]
assert sum(GROUPS) == BPC
# norm-square engine split per 16-image block: slots 0..NA-1 on ACT, the
# rest on DVE. Both engines' fused reduce ops run at 1x (no 2x uop exists
# for either; DVE STT+accum ~690ns, ACT Square+accum ~750ns solo but
# ~0.9us in situ), and DVE already carries the 64 dots. 13:3 measured
# best on hardware (56.3us/pass vs 60+ for 15:1 or 9:7).
NA = 13
ND = 16 - NA


def _cap_sync_waits(nc: bass.Bass, max_waits: int = 1) -> None:
    """The walrus build in this container encodes at most one sync-wait
    command per instruction ("Too many sync wait commands" in codegen
    otherwise), but Tile freely attaches several. Splitting the surplus
    waits onto single-wait Drain carriers right before the instruction is
    semantically identical: the engine blocks on each in turn.
    """
    for func in nc.m.functions:
        for bb in func.blocks:
            out = []
            for ins in bb.instructions:
                si = ins.sync_info
                if si is not None and len(si.on_wait) > max_waits:
                    waits = list(si.on_wait)
                    extra, keep = waits[:-max_waits], waits[-max_waits:]
                    for k, w in enumerate(extra):
                        d = mybir.InstDrain(
                            name=f"{ins.name}_w{k}",
                            ins=[],
                            outs=[],
                            engine=ins.engine,
                        )
                        d.sync_info = mybir.SyncInfo(on_wait=[w], on_update=[])
                        nc.register_instruction(d, overwrite=True)
                        out.append(d)
                    ins.sync_info = mybir.SyncInfo(
                        on_wait=keep, on_update=list(si.on_update)
                    )
                out.append(ins)
            bb.instructions = out


def build_nc(R: int = 1) -> bass.Bass:
    nc = bass.Bass(num_devices=NCORES)

    # text-major bf16 image block: imgT[a, b, d] = img[b, a, d], a<256
    imgT = nc.declare_dram_parameter("imgT", [BS, BPC, D], BF16, isOutput=False)
    # tail rows and false texts, (b, f)-packed: [p = (b*8+f) % 128, c, d]
    tailp = nc.declare_dram_parameter("tailp", [128, 2, D], BF16, isOutput=False)
    falsep = nc.declare_dram_parameter("falsep", [128, 2, D], BF16, isOutput=False)
    # rand text, a-chunked: randp[p, c, d] = rand[c*128+p, d]
    randp = nc.declare_dram_parameter("randp", [128, 2, D], BF16, isOutput=False)
    # one-hot (b,f)-row -> image map for the tail exp row sums
    wtail = nc.declare_dram_parameter("wtail", [128, 2, BPC], BF16, isOutput=False)
    # one-hot mask of this core's diagonal logits in column layout
    dmask = nc.declare_dram_parameter("dmask", [128, 2 * BPC], F32, isOutput=False)
    lscale = nc.declare_dram_parameter("lscale", [1], F32, isOutput=False)
    part_out = nc.declare_dram_parameter("part_out", [1, 2 * 128 + 1], F32, isOutput=True)

    with tile.TileContext(nc) as tc, ExitStack() as ctx:
        per = ctx.enter_context(tc.tile_pool(name="per", bufs=2))
        imgpool = ctx.enter_context(tc.tile_pool(name="img", bufs=4))
        tmppool = ctx.enter_context(tc.tile_pool(name="tmp", bufs=3))
        psum = ctx.enter_context(tc.tile_pool(name="psum", bufs=2, space="PSUM"))

        for _rep in range(R):
            # ---- preloads (ACT HWDGE ring for what ACT needs first; the
            # SWDGE ring for the rest; SP HWDGE carries only the stream) --
            ls_raw = per.tile([128, 1], F32, tag="ls_raw")
            nc.scalar.dma_start(out=ls_raw, in_=lscale[:].to_broadcast([128, 1]))
            rand2 = per.tile([128, 2, D], BF16, tag="rand2")
            nc.scalar.dma_start(out=rand2, in_=randp[:, :, :])
            tail_t = per.tile([128, 2, D], BF16, tag="tail_t")
            nc.gpsimd.dma_start(out=tail_t, in_=tailp[:, :, :])
            false_t = per.tile([128, 2, D], BF16, tag="false_t")
            nc.gpsimd.dma_start(out=false_t, in_=falsep[:, :, :])
            wt = per.tile([128, 2, BPC], BF16, tag="wt")
            nc.gpsimd.dma_start(out=wt, in_=wtail[:, :, :])
            dmk = per.tile([128, 2, BPC], F32, tag="dmk")
            nc.gpsimd.dma_start(
                out=dmk, in_=dmask[:, :].rearrange("p (c b) -> p c b", c=2)
            )

            scale_b = per.tile([128, 1], F32, tag="scale_b")
            nc.scalar.activation(scale_b, ls_raw, AF.Exp)
            ones_bf = per.tile([128, 1], BF16, tag="ones_bf")
            nc.vector.memset(ones_bf, 1.0)
            ones32 = per.tile([BPC, 1], F32, tag="ones32")
            nc.vector.memset(ones32, 1.0)
            neg2 = per.tile([128, 1], F32, tag="neg2")
            nc.vector.memset(neg2, -2.0)

            dots01 = per.tile([128, 2, BPC], F32, tag="dots01")
            # per-engine norm accumulators (image b -> block b//16, slot
            # b%16; slots < NA on ACT, others on DVE)
            nsqA = per.tile([128, 2, 2 * NA], F32, tag="nsqA")
            nsqD = per.tile([128, 2, 2 * ND], F32, tag="nsqD")

            # rand norms (ACT is free while the first img DMA streams)
            rn_sq = per.tile([128, 2], F32, tag="rn_sq")
            for c in range(2):
                sqr = tmppool.tile([128, D], BF16, tag="sqa")
                nc.scalar.activation(
                    sqr, rand2[:, c, :], AF.Square, accum_out=rn_sq[:, c : c + 1]
                )
            rn_isc = per.tile([128, 2], F32, tag="rn_isc")
            nc.scalar.activation(rn_isc, rn_sq, AF.Ln)
            nc.scalar.activation(rn_isc, rn_isc, AF.Exp, scale=-0.5)
            nc.vector.tensor_scalar_mul(rn_isc, rn_isc, scale_b)

            invA = per.tile([128, 2, 2 * NA], F32, tag="invA")
            invD = per.tile([128, 2, 2 * ND], F32, tag="invD")
            LB = per.tile([128, 2, BPC], F32, tag="LB")
            expLB = per.tile([128, 2, BPC], BF16, tag="expLB")
            cs = per.tile([128, 2], F32, tag="cs")
            dcol = per.tile([128, 2], F32, tag="dcol")
            # single PSUM bank accumulating every image's exp-row-sum
            rs_ps = psum.tile([BPC, 1], F32, tag="rs_ps")

            def dve_stt(in0, in1, acc, tag):
                o = tmppool.tile([128, D], BF16, tag=tag)
                nc.vector.scalar_tensor_tensor(
                    out=o, in0=in0, scalar=1.0, in1=in1,
                    op0=ALU.mult, op1=ALU.mult, accum_out=acc,
                )

            # ---- tail rows vs false texts (fills the DMA ramp) --------------
            ltr = per.tile([128, 2], F32, tag="ltr")
            nsq_tf = per.tile([128, 4], F32, tag="nsq_tf")
            for c in range(2):
                dve_stt(tail_t[:, c, :], false_t[:, c, :], ltr[:, c : c + 1], "sqd")
                sqa = tmppool.tile([128, D], BF16, tag="sqa")
                nc.scalar.activation(
                    sqa, tail_t[:, c, :], AF.Square, accum_out=nsq_tf[:, c : c + 1]
                )
                dve_stt(
                    false_t[:, c, :], false_t[:, c, :], nsq_tf[:, 2 + c : 3 + c],
                    "sqd",
                )
            # lt = ltr * rsqrt(|tail|^2 * |false|^2) * scale
            nn = per.tile([128, 2], F32, tag="nn")
            nc.vector.tensor_mul(nn, nsq_tf[:, 0:2], nsq_tf[:, 2:4])
            nc.scalar.activation(nn, nn, AF.Ln)
            nc.scalar.activation(nn, nn, AF.Exp, scale=-0.5)
            lt = per.tile([128, 2], F32, tag="lt")
            nc.vector.tensor_mul(lt, ltr, nn)
            nc.vector.tensor_scalar_mul(lt, lt, scale_b)
            exp_t = per.tile([128, 2], BF16, tag="exp_t")
            nc.scalar.activation(exp_t, lt, AF.Exp)
            # per-image tail exp sums, accumulated into the shared PSUM bank
            nc.tensor.matmul(rs_ps, wt[:, 0, :], exp_t[:, 0:1], start=True, stop=False)
            nc.tensor.matmul(rs_ps, wt[:, 1, :], exp_t[:, 1:2], start=False, stop=False)

            # ---- main stream (chunk-major): dots + squared norms ------------
            # (A 2x TT-multiply + bf16 add-tree + short segmented reduce was
            # tried for the dots and measured ~10us/pass SLOWER than the
            # fused 1x STT: the extra per-op DVE overhead outweighs the 2x
            # streaming rate. Keep the single fused op per image.)
            def do_cgroup(c, b0, gsz):
                src = imgT[c * 128 : (c + 1) * 128, b0 : b0 + gsz, :]
                img_t = imgpool.tile([128, gsz, D], BF16, tag=f"img{gsz}")
                nc.sync.dma_start(out=img_t, in_=src)
                for i in range(gsz):
                    b = b0 + i
                    ia = img_t[:, i, :]
                    dve_stt(ia, rand2[:, c, :], dots01[:, c, b : b + 1], "sqd")
                    blk, sl = b // 16, b % 16
                    if sl < NA:
                        ca = blk * NA + sl
                        sqa = tmppool.tile([128, D], BF16, tag="sqa")
                        nc.scalar.activation(
                            sqa, ia, AF.Square, accum_out=nsqA[:, c, ca : ca + 1]
                        )
                    else:
                        cd = blk * ND + (sl - NA)
                        dve_stt(ia, ia, nsqD[:, c, cd : cd + 1], "sqd")

            def post_chunk(c, stop):
                nc.scalar.activation(invA[:, c, :], nsqA[:, c, :], AF.Ln)
                nc.scalar.activation(invA[:, c, :], invA[:, c, :], AF.Exp, scale=-0.5)
                nc.scalar.activation(invD[:, c, :], nsqD[:, c, :], AF.Ln)
                nc.scalar.activation(invD[:, c, :], invD[:, c, :], AF.Exp, scale=-0.5)
                lbv = LB[:, c, :].rearrange("p (g e) -> p g e", e=16)
                dv = dots01[:, c, :].rearrange("p (g e) -> p g e", e=16)
                nc.vector.tensor_mul(
                    lbv[:, :, 0:NA], dv[:, :, 0:NA],
                    invA[:, c, :].rearrange("p (g e) -> p g e", e=NA),
                )
                nc.vector.tensor_mul(
                    lbv[:, :, NA:16], dv[:, :, NA:16],
                    invD[:, c, :].rearrange("p (g e) -> p g e", e=ND),
                )
                nc.vector.tensor_scalar_mul(
                    LB[:, c, :], LB[:, c, :], rn_isc[:, c : c + 1]
                )
                nc.scalar.activation(expLB[:, c, :], LB[:, c, :], AF.Exp)
                nc.vector.tensor_reduce(
                    cs[:, c : c + 1], expLB[:, c, :], axis=AX.X, op=ALU.add
                )
                # this chunk's share of the diagonal partial
                dprod = tmppool.tile([128, BPC], F32, tag="dprod")
                nc.vector.scalar_tensor_tensor(
                    out=dprod, in0=LB[:, c, :], scalar=1.0, in1=dmk[:, c, :],
                    op0=ALU.mult, op1=ALU.mult, accum_out=dcol[:, c : c + 1],
                )
                # this chunk's exp row sums -> shared PSUM bank
                nc.tensor.matmul(
                    rs_ps, expLB[:, c, :], ones_bf, start=False, stop=stop
                )

            for c in range(2):
                b0 = 0
                for gi, gsz in enumerate(GROUPS):
                    do_cgroup(c, b0, gsz)
                    b0 += gsz
                    if c == 1 and gi == 0:
                        post_chunk(0, stop=False)
                        nc.gpsimd.dma_start(
                            out=part_out[0:1, 0:128].rearrange("o p -> p o"),
                            in_=cs[:, 0:1],
                        )
            post_chunk(1, stop=True)

            lse = per.tile([BPC, 1], F32, tag="lse")
            nc.scalar.activation(lse, rs_ps, AF.Ln)
            dsum = per.tile([128, 1], F32, tag="dsum")
            nc.vector.tensor_add(dsum, dcol[:, 0:1], dcol[:, 1:2])

            # u = sum_i lse_i - 2 * sum diag  (single PSUM accumulation)
            u_ps = psum.tile([1, 1], F32, tag="usum")
            nc.tensor.matmul(u_ps, dsum, neg2, start=True, stop=False)
            nc.tensor.matmul(u_ps, lse, ones32, start=False, stop=True)
            uv2 = per.tile([1, 1], F32, tag="uv2")
            nc.scalar.copy(uv2, u_ps)

            # ---- write out this core's partials (host finishes the loss) ----
            nc.gpsimd.dma_start(
                out=part_out[0:1, 128:256].rearrange("o p -> p o"), in_=cs[:, 1:2]
            )
            nc.gpsimd.dma_start(out=part_out[0:1, 256:257], in_=uv2)

    _cap_sync_waits(nc)
    return nc


_NC = None


def _get_nc() -> bass.Bass:
    global _NC
    if _NC is None:
        _NC = build_nc(1)
    return _NC


BF = ml_dtypes.bfloat16


def make_in_maps(inputs: dict) -> list[dict]:
    img_full = np.asarray(inputs["image_features"], np.float32)
    rand = np.asarray(inputs["random_text_features"], np.float32)
    false = np.asarray(inputs["false_text_features"], np.float32)
    ls = np.asarray(inputs["logit_scale"], np.float32).reshape(1)

    randp = np.ascontiguousarray(
        rand.reshape(2, 128, D).transpose(1, 0, 2).astype(BF)
    )
    wt = np.zeros((128, 2, BPC), BF)
    for c in range(2):
        r = c * 128 + np.arange(128)
        wt[np.arange(128), c, r // FTN] = 1
    in_maps = []
    for m in range(NCORES):
        sl = slice(m * BPC, (m + 1) * BPC)
        imgT = np.ascontiguousarray(
            img_full[sl, :BS, :].transpose(1, 0, 2).astype(BF)
        )
        tailp = np.ascontiguousarray(
            img_full[sl, BS:ATN, :].reshape(2, 128, D).transpose(1, 0, 2).astype(BF)
        )
        falsep = np.ascontiguousarray(
            false[m * BPC * FTN : (m + 1) * BPC * FTN]
            .reshape(2, 128, D).transpose(1, 0, 2).astype(BF)
        )
        dm = np.zeros((128, 2 * BPC), np.float32)
        a = m * BPC + np.arange(BPC)
        dm[a % 128, (a // 128) * BPC + np.arange(BPC)] = 1.0
        in_maps.append(
            {
                "imgT": imgT,
                "tailp": tailp,
                "falsep": falsep,
                "randp": randp,
                "wtail": wt,
                "dmask": dm,
                "lscale": ls,
            }
        )
    return in_maps


def finish_loss(parts: np.ndarray) -> np.ndarray:
    """Combine the 8 per-core [257] partials into the scalar loss.

    parts[m, a<256]: core m's partial column sum of exp(logits) for text a
    parts[m, 256]:   core m's (sum_i lse_i - 2*sum_i diag_i)
    """
    parts = np.asarray(parts, np.float32).reshape(NCORES, 2 * 128 + 1)
    colsum = parts[:, 0:256].sum(axis=0)
    u = parts[:, 256].sum()
    return np.float32((u + np.log(colsum).sum()) / (2.0 * BS)).reshape(())


def kernel(**inputs) -> np.ndarray:
    nc = _get_nc()
    res = run_bass_kernel_spmd(nc, make_in_maps(inputs), list(range(NCORES)))
    parts = np.stack(
        [np.asarray(r["part_out"], np.float32).reshape(-1) for r in res.results]
    )
    return finish_loss(parts)


# revision 14
# speedup vs baseline: 4.4070x; 1.8269x over previous
"""Trainium2 Bass kernel for the 27092653703365 contrastive loss.

Strategy (memory-bound; the [256, 264, 512] image block dominates):
  - Data-parallel shard of the batch dim (bs=256) across 8 NeuronCores
    (32 images per core); random_text_features replicated.
  - Sharding-time prep (host, one-time): each core's image block is cast
    to bf16 and laid out text-major ([a=256, b=32, d=512] contiguous), so
    every bulk DMA is a plain contiguous HWDGE load (128 descriptors x
    8KB) and HBM traffic halves vs f32. bf16 logits keep the loss within
    ~2e-7 of f32 (gate is 2e-2). Tail rows (a=256..263) and false texts
    are packed [(b,f)=128 partitions, 2, 512] so the tail costs 6 wide
    ops instead of 24 narrow ones.
  - Per core: stream the 8.4MB bf16 block through SBUF once. Each
    (image, text-row) needs its dot with one text vector plus its
    squared norm: DVE does all dots (STT+accum, 2x bf16 mode) and 7/16
    of the squares; ACT does the other 9/16 (Square+accum). The norm
    accumulators are split per engine (nsqA/nsqD) so ACT and DVE never
    ping-pong write the same tile; outputs leave on the idle SWDGE ring
    so the sync HWDGE FIFO only ever carries the image stream.
  - Row sums of exp(logits) go through PE ones-matmuls accumulated in a
    single PSUM bank (no transposes); all activations stay on the
    natural_log_exp table set (1/sqrt as exp(-0.5 ln)).
  - No on-device collective: an 8-byte AllGather alone measures ~140us
    on this runtime (trigger/rendezvous dominated), so each core returns
    its 257-float partial (column sums of exp(logits) for its images +
    the row-CE partial) and kernel() finishes the scalar loss on the
    host while unsharding -- a ~2KB numpy epilogue.

build_nc(R) emits R identical back-to-back passes of the workload in one
NEFF; kernel() runs R=1. The replicas exist so the test harness can time
the kernel far above the shared axon tunnel's per-call dispatch noise.
"""

import sys

sys.path.insert(0, "/opt/trn_rl_repo")

from contextlib import ExitStack

import ml_dtypes
import numpy as np

import concourse.bass as bass
import concourse.tile as tile
from concourse import mybir
from concourse.bass_utils import run_bass_kernel_spmd

F32 = mybir.dt.float32
BF16 = mybir.dt.bfloat16
AF = mybir.ActivationFunctionType
ALU = mybir.AluOpType
AX = mybir.AxisListType

NCORES = 8
BS, FTN, D = 256, 8, 512
ATN = BS + FTN  # 264
BPC = BS // NCORES  # 32 images per core
# image-group sizes per 128-text chunk: small first groups shorten the
# DMA ramp so compute starts early
GROUPS = [4, 4, 8, 8, 8]
assert sum(GROUPS) == BPC
# norm-square engine split per 16-image block: slots 0..NA-1 on ACT, the
# rest on DVE. Both engines' fused reduce ops run at 1x (no 2x uop exists
# for either; DVE STT+accum ~690ns, ACT Square+accum ~750ns solo but
# ~0.9us in situ), and DVE already carries the 64 dots. 13:3 measured
# best on hardware (56.3us/pass vs 60+ for 15:1 or 9:7).
NA = 13
ND = 16 - NA


def _cap_sync_waits(nc: bass.Bass, max_waits: int = 1) -> None:
    """The walrus build in this container encodes at most one sync-wait
    command per instruction ("Too many sync wait commands" in codegen
    otherwise), but Tile freely attaches several. Splitting the surplus
    waits onto single-wait Drain carriers right before the instruction is
    semantically identical: the engine blocks on each in turn.
    """
    for func in nc.m.functions:
        for bb in func.blocks:
            out = []
            for ins in bb.instructions:
                si = ins.sync_info
                if si is not None and len(si.on_wait) > max_waits:
                    waits = list(si.on_wait)
                    extra, keep = waits[:-max_waits], waits[-max_waits:]
                    for k, w in enumerate(extra):
                        d = mybir.InstDrain(
                            name=f"{ins.name}_w{k}",
                            ins=[],
                            outs=[],
                            engine=ins.engine,
                        )
                        d.sync_info = mybir.SyncInfo(on_wait=[w], on_update=[])
                        nc.register_instruction(d, overwrite=True)
                        out.append(d)
                    ins.sync_info = mybir.SyncInfo(
                        on_wait=keep, on_update=list(si.on_update)
                    )
                out.append(ins)
            bb.instructions = out


def build_nc(R: int = 1) -> bass.Bass:
    nc = bass.Bass(num_devices=NCORES)

    # text-major bf16 image block: imgT[a, b, d] = img[b, a, d], a<256
    imgT = nc.declare_dram_parameter("imgT", [BS, BPC, D], BF16, isOutput=False)
    # tail rows and false texts, (b, f)-packed: [p = (b*8+f) % 128, c, d]
    tailp = nc.declare_dram_parameter("tailp", [128, 2, D], BF16, isOutput=False)
    falsep = nc.declare_dram_parameter("falsep", [128, 2, D], BF16, isOutput=False)
    # rand text, a-chunked: randp[p, c, d] = rand[c*128+p, d]
    randp = nc.declare_dram_parameter("randp", [128, 2, D], BF16, isOutput=False)
    # one-hot (b,f)-row -> image map for the tail exp row sums
    wtail = nc.declare_dram_parameter("wtail", [128, 2, BPC], BF16, isOutput=False)
    # one-hot mask of this core's diagonal logits in column layout
    dmask = nc.declare_dram_parameter("dmask", [128, 2 * BPC], F32, isOutput=False)
    lscale = nc.declare_dram_parameter("lscale", [1], F32, isOutput=False)
    part_out = nc.declare_dram_parameter("part_out", [1, 2 * 128 + 1], F32, isOutput=True)

    with tile.TileContext(nc) as tc, ExitStack() as ctx:
        per = ctx.enter_context(tc.tile_pool(name="per", bufs=2))
        imgpool = ctx.enter_context(tc.tile_pool(name="img", bufs=4))
        tmppool = ctx.enter_context(tc.tile_pool(name="tmp", bufs=3))
        psum = ctx.enter_context(tc.tile_pool(name="psum", bufs=2, space="PSUM"))

        for _rep in range(R):
            # ---- preloads (ACT HWDGE ring for what ACT needs first; the
            # SWDGE ring for the rest; SP HWDGE carries only the stream) --
            ls_raw = per.tile([128, 1], F32, tag="ls_raw")
            nc.scalar.dma_start(out=ls_raw, in_=lscale[:].to_broadcast([128, 1]))
            rand2 = per.tile([128, 2, D], BF16, tag="rand2")
            nc.scalar.dma_start(out=rand2, in_=randp[:, :, :])
            tail_t = per.tile([128, 2, D], BF16, tag="tail_t")
            nc.gpsimd.dma_start(out=tail_t, in_=tailp[:, :, :])
            false_t = per.tile([128, 2, D], BF16, tag="false_t")
            nc.gpsimd.dma_start(out=false_t, in_=falsep[:, :, :])
            wt = per.tile([128, 2, BPC], BF16, tag="wt")
            nc.gpsimd.dma_start(out=wt, in_=wtail[:, :, :])
            dmk = per.tile([128, 2, BPC], F32, tag="dmk")
            nc.gpsimd.dma_start(
                out=dmk, in_=dmask[:, :].rearrange("p (c b) -> p c b", c=2)
            )

            ones_bf = per.tile([128, 1], BF16, tag="ones_bf")
            nc.vector.memset(ones_bf, 1.0)
            ones32 = per.tile([BPC, 1], F32, tag="ones32")
            nc.vector.memset(ones32, 1.0)
            neg2 = per.tile([128, 1], F32, tag="neg2")
            nc.vector.memset(neg2, -2.0)

            dots01 = per.tile([128, 2, BPC], F32, tag="dots01")
            # per-engine norm accumulators (image b -> block b//16, slot
            # b%16; slots < NA on ACT, others on DVE)
            nsqA = per.tile([128, 2, 2 * NA], F32, tag="nsqA")
            nsqD = per.tile([128, 2, 2 * ND], F32, tag="nsqD")

            # rand norms (ACT is free while the first img DMA streams)
            rn_sq = per.tile([128, 2], F32, tag="rn_sq")
            for c in range(2):
                sqr = tmppool.tile([128, D], BF16, tag="sqa")
                nc.scalar.activation(
                    sqr, rand2[:, c, :], AF.Square, accum_out=rn_sq[:, c : c + 1]
                )
            # lnrs = -0.5*ln(|rand|^2) + ln(scale); folded as the bias of the
            # image-norm Exp so L = dots * exp(-0.5*ln(nsq) + lnrs) needs no
            # separate rand-norm or logit-scale multiplies (lscale IS
            # ln(scale), so no Exp of it is needed either).
            lnrs = per.tile([128, 2], F32, tag="lnrs")
            nc.scalar.activation(lnrs, rn_sq, AF.Ln)
            nc.vector.tensor_scalar(
                out=lnrs, in0=lnrs, scalar1=-0.5, scalar2=ls_raw,
                op0=ALU.mult, op1=ALU.add,
            )

            invA = per.tile([128, 2, 2 * NA], F32, tag="invA")
            invD = per.tile([128, 2, 2 * ND], F32, tag="invD")
            LB = per.tile([128, 2, BPC], F32, tag="LB")
            expLB = per.tile([128, 2, BPC], BF16, tag="expLB")
            cs = per.tile([128, 2], F32, tag="cs")
            dcol = per.tile([128, 2], F32, tag="dcol")
            # single PSUM bank accumulating every image's exp-row-sum
            rs_ps = psum.tile([BPC, 1], F32, tag="rs_ps")

            def dve_stt(in0, in1, acc, tag):
                o = tmppool.tile([128, D], BF16, tag=tag)
                nc.vector.scalar_tensor_tensor(
                    out=o, in0=in0, scalar=1.0, in1=in1,
                    op0=ALU.mult, op1=ALU.mult, accum_out=acc,
                )

            # ---- tail rows vs false texts (fills the DMA ramp) --------------
            ltr = per.tile([128, 2], F32, tag="ltr")
            nsq_tf = per.tile([128, 4], F32, tag="nsq_tf")
            for c in range(2):
                dve_stt(tail_t[:, c, :], false_t[:, c, :], ltr[:, c : c + 1], "sqd")
                sqa = tmppool.tile([128, D], BF16, tag="sqa")
                nc.scalar.activation(
                    sqa, tail_t[:, c, :], AF.Square, accum_out=nsq_tf[:, c : c + 1]
                )
                dve_stt(
                    false_t[:, c, :], false_t[:, c, :], nsq_tf[:, 2 + c : 3 + c],
                    "sqd",
                )
            # lt = ltr * exp(-0.5*ln(|tail|^2 |false|^2) + ln(scale))
            nn = per.tile([128, 2], F32, tag="nn")
            nc.vector.tensor_mul(nn, nsq_tf[:, 0:2], nsq_tf[:, 2:4])
            nc.scalar.activation(nn, nn, AF.Ln)
            nc.scalar.activation(nn, nn, AF.Exp, scale=-0.5, bias=ls_raw)
            lt = per.tile([128, 2], F32, tag="lt")
            nc.vector.tensor_mul(lt, ltr, nn)
            exp_t = per.tile([128, 2], BF16, tag="exp_t")
            nc.scalar.activation(exp_t, lt, AF.Exp)
            # per-image tail exp sums, accumulated into the shared PSUM bank
            nc.tensor.matmul(rs_ps, wt[:, 0, :], exp_t[:, 0:1], start=True, stop=False)
            nc.tensor.matmul(rs_ps, wt[:, 1, :], exp_t[:, 1:2], start=False, stop=False)

            # ---- main stream (chunk-major): dots + squared norms ------------
            # (A 2x TT-multiply + bf16 add-tree + short segmented reduce was
            # tried for the dots and measured ~10us/pass SLOWER than the
            # fused 1x STT: the extra per-op DVE overhead outweighs the 2x
            # streaming rate. Keep the single fused op per image.)
            def do_cgroup(c, b0, gsz):
                src = imgT[c * 128 : (c + 1) * 128, b0 : b0 + gsz, :]
                img_t = imgpool.tile([128, gsz, D], BF16, tag=f"img{gsz}")
                nc.sync.dma_start(out=img_t, in_=src)
                for i in range(gsz):
                    b = b0 + i
                    ia = img_t[:, i, :]
                    dve_stt(ia, rand2[:, c, :], dots01[:, c, b : b + 1], "sqd")
                    blk, sl = b // 16, b % 16
                    if sl < NA:
                        ca = blk * NA + sl
                        sqa = tmppool.tile([128, D], BF16, tag="sqa")
                        nc.scalar.activation(
                            sqa, ia, AF.Square, accum_out=nsqA[:, c, ca : ca + 1]
                        )
                    else:
                        cd = blk * ND + (sl - NA)
                        dve_stt(ia, ia, nsqD[:, c, cd : cd + 1], "sqd")

            def post_chunk(c, stop):
                nc.scalar.activation(invA[:, c, :], nsqA[:, c, :], AF.Ln)
                nc.scalar.activation(
                    invA[:, c, :], invA[:, c, :], AF.Exp, scale=-0.5,
                    bias=lnrs[:, c : c + 1],
                )
                nc.scalar.activation(invD[:, c, :], nsqD[:, c, :], AF.Ln)
                nc.scalar.activation(
                    invD[:, c, :], invD[:, c, :], AF.Exp, scale=-0.5,
                    bias=lnrs[:, c : c + 1],
                )
                lbv = LB[:, c, :].rearrange("p (g e) -> p g e", e=16)
                dv = dots01[:, c, :].rearrange("p (g e) -> p g e", e=16)
                nc.vector.tensor_mul(
                    lbv[:, :, 0:NA], dv[:, :, 0:NA],
                    invA[:, c, :].rearrange("p (g e) -> p g e", e=NA),
                )
                nc.vector.tensor_mul(
                    lbv[:, :, NA:16], dv[:, :, NA:16],
                    invD[:, c, :].rearrange("p (g e) -> p g e", e=ND),
                )
                # the exp's free accumulator doubles as the column sum
                nc.scalar.activation(
                    expLB[:, c, :], LB[:, c, :], AF.Exp,
                    accum_out=cs[:, c : c + 1],
                )
                # this chunk's share of the diagonal partial
                dprod = tmppool.tile([128, BPC], F32, tag="dprod")
                nc.vector.scalar_tensor_tensor(
                    out=dprod, in0=LB[:, c, :], scalar=1.0, in1=dmk[:, c, :],
                    op0=ALU.mult, op1=ALU.mult, accum_out=dcol[:, c : c + 1],
                )
                # this chunk's exp row sums -> shared PSUM bank
                nc.tensor.matmul(
                    rs_ps, expLB[:, c, :], ones_bf, start=False, stop=stop
                )

            for c in range(2):
                b0 = 0
                for gi, gsz in enumerate(GROUPS):
                    do_cgroup(c, b0, gsz)
                    b0 += gsz
                    if c == 1 and gi == 0:
                        post_chunk(0, stop=False)
                        nc.gpsimd.dma_start(
                            out=part_out[0:1, 0:128].rearrange("o p -> p o"),
                            in_=cs[:, 0:1],
                        )
            post_chunk(1, stop=True)

            lse = per.tile([BPC, 1], F32, tag="lse")
            nc.scalar.activation(lse, rs_ps, AF.Ln)
            dsum = per.tile([128, 1], F32, tag="dsum")
            nc.vector.tensor_add(dsum, dcol[:, 0:1], dcol[:, 1:2])

            # u = sum_i lse_i - 2 * sum diag  (single PSUM accumulation)
            u_ps = psum.tile([1, 1], F32, tag="usum")
            nc.tensor.matmul(u_ps, dsum, neg2, start=True, stop=False)
            nc.tensor.matmul(u_ps, lse, ones32, start=False, stop=True)
            uv2 = per.tile([1, 1], F32, tag="uv2")
            nc.scalar.copy(uv2, u_ps)

            # ---- write out this core's partials (host finishes the loss) ----
            nc.gpsimd.dma_start(
                out=part_out[0:1, 128:256].rearrange("o p -> p o"), in_=cs[:, 1:2]
            )
            nc.gpsimd.dma_start(out=part_out[0:1, 256:257], in_=uv2)

    _cap_sync_waits(nc)
    return nc


_NC = None


def _get_nc() -> bass.Bass:
    global _NC
    if _NC is None:
        _NC = build_nc(1)
    return _NC


BF = ml_dtypes.bfloat16


def make_in_maps(inputs: dict) -> list[dict]:
    img_full = np.asarray(inputs["image_features"], np.float32)
    rand = np.asarray(inputs["random_text_features"], np.float32)
    false = np.asarray(inputs["false_text_features"], np.float32)
    ls = np.asarray(inputs["logit_scale"], np.float32).reshape(1)

    randp = np.ascontiguousarray(
        rand.reshape(2, 128, D).transpose(1, 0, 2).astype(BF)
    )
    wt = np.zeros((128, 2, BPC), BF)
    for c in range(2):
        r = c * 128 + np.arange(128)
        wt[np.arange(128), c, r // FTN] = 1
    in_maps = []
    for m in range(NCORES):
        sl = slice(m * BPC, (m + 1) * BPC)
        imgT = np.ascontiguousarray(
            img_full[sl, :BS, :].transpose(1, 0, 2).astype(BF)
        )
        tailp = np.ascontiguousarray(
            img_full[sl, BS:ATN, :].reshape(2, 128, D).transpose(1, 0, 2).astype(BF)
        )
        falsep = np.ascontiguousarray(
            false[m * BPC * FTN : (m + 1) * BPC * FTN]
            .reshape(2, 128, D).transpose(1, 0, 2).astype(BF)
        )
        dm = np.zeros((128, 2 * BPC), np.float32)
        a = m * BPC + np.arange(BPC)
        dm[a % 128, (a // 128) * BPC + np.arange(BPC)] = 1.0
        in_maps.append(
            {
                "imgT": imgT,
                "tailp": tailp,
                "falsep": falsep,
                "randp": randp,
                "wtail": wt,
                "dmask": dm,
                "lscale": ls,
            }
        )
    return in_maps


def finish_loss(parts: np.ndarray) -> np.ndarray:
    """Combine the 8 per-core [257] partials into the scalar loss.

    parts[m, a<256]: core m's partial column sum of exp(logits) for text a
    parts[m, 256]:   core m's (sum_i lse_i - 2*sum_i diag_i)
    """
    parts = np.asarray(parts, np.float32).reshape(NCORES, 2 * 128 + 1)
    colsum = parts[:, 0:256].sum(axis=0)
    u = parts[:, 256].sum()
    return np.float32((u + np.log(colsum).sum()) / (2.0 * BS)).reshape(())


def kernel(**inputs) -> np.ndarray:
    nc = _get_nc()
    res = run_bass_kernel_spmd(nc, make_in_maps(inputs), list(range(NCORES)))
    parts = np.stack(
        [np.asarray(r["part_out"], np.float32).reshape(-1) for r in res.results]
    )
    return finish_loss(parts)
